# revision 5
# baseline (speedup 1.0000x reference)
"""Trainium2 Bass kernel for nn_Aggregation0 (fold -> normalize -> unfold).

Per (image, hor_f) slice the op is: col2im (5x5, stride 1) of the 25
ver_f channels into a 64x64 image, divide by the overlap count, then
im2col back. The output is therefore 25 shifted views of the folded
image. Sharding: one image per NeuronCore (8 images, 8 cores).

Layout per core: x_im (3600, 1600) f32 where 1600 = (h=64, v=25) is
contiguous per patch p = (qi, qj) in a 60x60 grid.
  Phase 1 (PE): per 120-partition tile (2 qi rows), contract qj with 5
    column-shift matrices -> Yc[(qi_r, j); (ei, h)] in PSUM.
  Phase 2 (PE): contract qi/ei with identity matmuls, accumulating the
    folded image img[(r, j); (i2, h)] in PSUM (i = 2*i2 + r).
  Normalize (DVE): multiply by precomputed 1/count.
  Shifts (PE): 4 column-shifted copies of img so that unfold reads are
    quadrant-aligned in the partition dimension.
  Phase 3 (DVE+ACT): 15 strided copies per output tile gather the 25
    shifted views into (h, v) layout; DMA out.
"""

import numpy as np

IMAGES = 8
PATCHES = 3600
HF = 64  # hor_f
VF = 25  # ver_f = 5*5
KP = 5  # patch width
OW = 60  # output patch grid (60x60)
IH = 64  # image height/width
FREE = HF * VF  # 1600
NT = 30  # partition tiles per image
TP = 120  # partitions per tile (2 qi rows x 60 qj)

_CACHE = {}


def _consts():
    c1 = np.array(
        [min(i, OW - 1) - max(i - (KP - 1), 0) + 1 for i in range(IH)],
        np.float32,
    )

    wc = np.zeros((TP, 5 * 128), np.float32)
    for ej in range(KP):
        for r in range(2):
            for qj in range(OW):
                j = qj + ej
                wc[r * OW + qj, ej * 128 + r * 64 + j] = 1.0

    id128 = np.eye(128, dtype=np.float32)

    id2 = np.zeros((128, 64), np.float32)
    for k in range(128):
        id2[k, k % 64] = 1.0

    shift = np.zeros((128, 4 * 128), np.float32)
    for dj in range(1, KP):
        for r in range(2):
            for j in range(IH - dj):
                shift[r * 64 + j + dj, (dj - 1) * 128 + r * 64 + j] = 1.0

    recip = np.zeros((128, 2048), np.float32)
    for r in range(2):
        for j in range(64):
            for i2 in range(32):
                recip[r * 64 + j, i2 * 64:(i2 + 1) * 64] = 1.0 / (
                    c1[2 * i2 + r] * c1[j]
                )
    return wc, id128, id2, shift, recip


def _build_nc(dbg=False):
    import concourse.bacc as bacc
    import concourse.mybir as mybir
    from concourse.tile import TileContext

    f32 = mybir.dt.float32
    nc = bacc.Bacc("TRN2", target_bir_lowering=False, debug=False)
    x = nc.dram_tensor("x", [PATCHES, FREE], f32, kind="ExternalInput")
    y = nc.dram_tensor("y", [PATCHES, FREE], f32, kind="ExternalOutput")
    if dbg:
        dbg_yc = nc.dram_tensor("dbg_yc", [NT, 128, 320], f32,
                                kind="ExternalOutput")
        dbg_img = nc.dram_tensor("dbg_img", [KP, 128, 2048], f32,
                                 kind="ExternalOutput")

    wc_np, id128_np, id2_np, shift_np, recip_np = _consts()
    wc_d = nc.inline_tensor(wc_np, name="wc_c")
    id128_d = nc.inline_tensor(id128_np, name="id128_c")
    id2_d = nc.inline_tensor(id2_np, name="id2_c")
    shift_d = nc.inline_tensor(shift_np, name="shift_c")
    recip_d = nc.inline_tensor(recip_np, name="recip_c")

    # engine split for phase-3 tiles (DVE ~3.2us/tile vs ACT ~4.7us/tile)
    dve_tiles = set(b for b in range(NT) if b % 5 < 3)

    with TileContext(nc) as tc:
        with (
            tc.tile_pool(name="const", bufs=1) as cpool,
            tc.tile_pool(name="ycsb", bufs=NT) as ycsb_pool,
            tc.tile_pool(name="imgsb", bufs=5) as img_pool,
            tc.tile_pool(name="inp", bufs=4) as in_pool,
            tc.tile_pool(name="outp", bufs=4) as out_pool,
            tc.tile_pool(name="ycps", bufs=2, space="PSUM") as ycps_pool,
            tc.tile_pool(name="imgps", bufs=3, space="PSUM") as imgps_pool,
        ):
            wc_sb = cpool.tile([TP, 5 * 128], f32)
            id128_sb = cpool.tile([128, 128], f32)
            id2_sb = cpool.tile([128, 64], f32)
            shift_sb = cpool.tile([128, 4 * 128], f32)
            recip_sb = cpool.tile([128, 2048], f32)
            zeros_sb = cpool.tile([128, 512], f32)
            nc.sync.dma_start(out=wc_sb[:], in_=wc_d[:])
            nc.sync.dma_start(out=id128_sb[:], in_=id128_d[:])
            nc.sync.dma_start(out=id2_sb[:], in_=id2_d[:])
            nc.sync.dma_start(out=shift_sb[:], in_=shift_d[:])
            nc.sync.dma_start(out=recip_sb[:], in_=recip_d[:])
            nc.vector.memset(zeros_sb[:], 0.0)

            # ---- Phase 1: column fold per tile (PE), drain Yc to SBUF ----
            yc_sb = []
            for b in range(NT):
                in_t = in_pool.tile([TP, FREE], f32, tag="in_t")
                nc.sync.dma_start(out=in_t[:], in_=x[b * TP:(b + 1) * TP, :])
                in_r = in_t[:].rearrange(
                    "p (h ei ej) -> p ei h ej", h=HF, ei=KP, ej=KP
                )
                yc_ps = ycps_pool.tile([128, 320], f32, tag="yc_ps")
                for ej in range(KP):
                    nc.tensor.matmul(
                        yc_ps[:, :],
                        lhsT=wc_sb[:, ej * 128:(ej + 1) * 128],
                        rhs=in_r[:, :, :, ej],
                        start=(ej == 0),
                        stop=(ej == KP - 1),
                    )
                yc = ycsb_pool.tile([128, 320], f32, tag="yc", name=f"yc{b}")
                eng = nc.vector if b % 2 == 0 else nc.scalar
                if eng is nc.vector:
                    eng.tensor_copy(out=yc[:], in_=yc_ps[:])
                else:
                    eng.copy(out=yc[:], in_=yc_ps[:])
                if dbg:
                    nc.sync.dma_start(out=dbg_yc[b], in_=yc[:])
                yc_sb.append(yc)

            # ---- Phase 2: row fold, accumulate img in PSUM (PE) ----
            # img half-tiles: imgA = i2 0..15, imgB = i2 16..31 (2 banks each)
            items_by_bank = {0: [], 1: [], 2: [], 3: []}
            for b in range(NT):
                for ei in range(KP):
                    if ei % 2 == 0:
                        i2 = b + ei // 2
                        items_by_bank[i2 // 8].append((b, ei, None, i2))
                    else:
                        for rho in (0, 1):
                            i2 = b + (rho + ei) // 2
                            items_by_bank[i2 // 8].append((b, ei, rho, i2))

            img_halves = []
            for h in range(2):
                img_ps = imgps_pool.tile(
                    [128, 1024], f32, tag="imgps", name=f"imgps{h}"
                )
                img_halves.append(img_ps)
            for g in range(4):
                half = img_halves[g // 2]
                base_i2 = (g // 2) * 16
                # zero-init the bank: sets has_written on all 512 elements
                bank_off = (g % 2) * 512
                nc.tensor.matmul(
                    half[:, bank_off:bank_off + 512],
                    lhsT=id128_sb[:],
                    rhs=zeros_sb[:],
                    start=True,
                    stop=False,
                    skip_group_check=True,
                )
                items = items_by_bank[g]
                for idx, (b, ei, rho, i2) in enumerate(items):
                    loc = (i2 - base_i2) * 64
                    sp = idx == len(items) - 1
                    if rho is None:
                        nc.tensor.matmul(
                            half[:, loc:loc + 64],
                            lhsT=id128_sb[:],
                            rhs=yc_sb[b][:, ei * 64:(ei + 1) * 64],
                            start=False,
                            stop=sp,
                            skip_group_check=True,
                        )
                    else:
                        dr = 1 - rho
                        nc.tensor.matmul(
                            half[dr * 64:(dr + 1) * 64, loc:loc + 64],
                            lhsT=id2_sb[rho * 64:(rho + 1) * 64, :],
                            rhs=yc_sb[b][rho * 64:(rho + 1) * 64,
                                         ei * 64:(ei + 1) * 64],
                            start=False,
                            stop=sp,
                            skip_group_check=True,
                        )

            # ---- Normalize + 4 shifted copies ----
            img_sb = []
            for dj in range(KP):
                t = img_pool.tile([128, 2048], f32, tag="img", name=f"img{dj}")
                img_sb.append(t)
            for h in range(2):
                nc.vector.tensor_mul(
                    out=img_sb[0][:, h * 1024:(h + 1) * 1024],
                    in0=img_halves[h][:],
                    in1=recip_sb[:, h * 1024:(h + 1) * 1024],
                )
            for dj in range(1, KP):
                for h in range(2):
                    sh_ps = imgps_pool.tile(
                        [128, 1024], f32, tag="imgps", name=f"shps{dj}_{h}"
                    )
                    for q in range(2):
                        nc.tensor.matmul(
                            sh_ps[:, q * 512:(q + 1) * 512],
                            lhsT=shift_sb[:, (dj - 1) * 128:dj * 128],
                            rhs=img_sb[0][:, h * 1024 + q * 512:
                                          h * 1024 + (q + 1) * 512],
                            start=True,
                            stop=True,
                        )
                    eng = nc.vector if (dj + h) % 2 == 0 else nc.scalar
                    dst = img_sb[dj][:, h * 1024:(h + 1) * 1024]
                    if eng is nc.vector:
                        eng.tensor_copy(out=dst, in_=sh_ps[:])
                    else:
                        eng.copy(out=dst, in_=sh_ps[:])

            if dbg:
                for dj in range(KP):
                    nc.sync.dma_start(out=dbg_img[dj], in_=img_sb[dj][:])

            # ---- Phase 3: unfold (gather 25 shifted views) + DMA out ----
            for b in range(NT):
                eng = nc.vector if b in dve_tiles else nc.scalar
                out_t = out_pool.tile([128, FREE], f32, tag="out_t")
                de = out_t[:].rearrange(
                    "p (h d5 dj) -> p d5 h dj", h=HF, d5=KP, dj=KP
                )
                for dj in range(KP):
                    src = img_sb[dj][:].rearrange("p (i2 h) -> p i2 h", i2=32)
                    # di even: both r groups in one op (partitions 0..123)
                    dst_e = de[0:124, 0:KP:2, :, dj]
                    src_e = src[0:124, b:b + 3, :]
                    if eng is nc.vector:
                        eng.tensor_copy(out=dst_e, in_=src_e)
                    else:
                        eng.copy(out=dst_e, in_=src_e)
                    # di odd: r groups separately (partition base 0 / 64)
                    # i2 = b + (rho + di)//2 for di in {1,3}:
                    #   rho=0 -> {b, b+1}; rho=1 -> {b+1, b+2}
                    for rho in (0, 1):
                        i2lo = b + rho
                        dst_o = de[rho * 64:rho * 64 + 60, 1:KP:2, :, dj]
                        src_o = src[(1 - rho) * 64:(1 - rho) * 64 + 60,
                                    i2lo:i2lo + 2, :]
                        if eng is nc.vector:
                            eng.tensor_copy(out=dst_o, in_=src_o)
                        else:
                            eng.copy(out=dst_o, in_=src_o)
                # NOTE: a single DMA with a 2-level partition AP misbehaves
                # on HW (probe3.py); use two half-tile DMAs. They hit
                # disjoint SDMA engine sets (partitions 0-59 vs 64-123).
                nc.scalar.dma_start(
                    out=y[b * TP:b * TP + OW, :], in_=out_t[0:OW, :]
                )
                nc.scalar.dma_start(
                    out=y[b * TP + OW:(b + 1) * TP, :],
                    in_=out_t[64:64 + OW, :],
                )

    nc.compile()
    return nc


def _get_nc():
    if "nc" not in _CACHE:
        _CACHE["nc"] = _build_nc()
    return _CACHE["nc"]


def kernel(x, pixels_h=64, pixels_w=64, **kw):
    from concourse.bass_utils import run_bass_kernel_spmd

    x = np.asarray(x, dtype=np.float32)
    assert x.shape == (IMAGES, PATCHES, HF, VF), x.shape
    nc = _get_nc()
    in_maps = [
        {"x": np.ascontiguousarray(x[im].reshape(PATCHES, FREE))}
        for im in range(IMAGES)
    ]
    res = run_bass_kernel_spmd(
        nc, in_maps, core_ids=list(range(IMAGES)), **kw
    )
    out = np.stack(
        [res.results[c]["y"].reshape(PATCHES, HF, VF) for c in range(IMAGES)]
    )
    if kw.get("trace"):
        kernel.last_results = res
    return out


# revision 10
# speedup vs baseline: 1.9406x; 1.9406x over previous
"""Trainium2 Bass kernel for nn_Aggregation0 (fold -> normalize -> unfold).

Per (image, hor_f) slice the op is: col2im (5x5, stride 1) of the 25
ver_f channels into a 64x64 image, divide by the overlap count, then
im2col back. The output is 25 shifted views of the folded image.
Sharding: one image per NeuronCore (8 images, 8 cores).

Host side: x[im] is re-packed to (p, {hi,lo}, ej, ei, h) bf16, where
x = hi + lo (bf16 split, exact to ~1e-5 rel). This makes the phase-1
matmul moving operand contiguous and halves input DMA bytes.

Per core:
  Phase 1 (PE, bf16): per 120-partition tile (2 qi rows of the 60x60
    patch grid), contract qj with 5 column-shift matrices (hi+lo
    accumulated in fp32 PSUM) -> Yc[(qi_r, j); (ei, h)].
  Phase 2 (DVE/ACT): windowed adds of Yc (read from PSUM) into the
    folded image img_raw[(r, j); (i2, h)] in SBUF (i = 2*i2 + r).
  Normalize (DVE/ACT): img0 = img_raw * (1/count), per 512-col quarter.
  Shifts (PE, fp32): 4 column-shifted copies of img0 so unfold reads
    are quadrant-aligned in the partition dim; drained per quarter.
  Phase 3 (DVE/ACT/GPSIMD): 35 2-D strided copies per output tile
    gather the 25 shifted views into (h, v) layout; 2 DMAs out.
"""

import numpy as np

IMAGES = 8
PATCHES = 3600
HF = 64  # hor_f
VF = 25  # ver_f = 5*5
KP = 5  # patch width
OW = 60  # output patch grid (60x60)
IH = 64  # image height/width
FREE = HF * VF  # 1600
NT = 30  # partition tiles per image
TP = 120  # partitions per tile (2 qi rows x 60 qj)

_CACHE = {}

# phase-3 engine split: tile -> engine kind
P3_DVE = 14
P3_ACT = 9  # remainder goes to gpsimd


def _consts():
    c1 = np.array(
        [min(i, OW - 1) - max(i - (KP - 1), 0) + 1 for i in range(IH)],
        np.float32,
    )

    wc = np.zeros((TP, 5 * 128), np.float32)
    for ej in range(KP):
        for r in range(2):
            for qj in range(OW):
                j = qj + ej
                wc[r * OW + qj, ej * 128 + r * 64 + j] = 1.0

    shift = np.zeros((128, 4 * 128), np.float32)
    for dj in range(1, KP):
        for r in range(2):
            for j in range(IH - dj):
                shift[r * 64 + j + dj, (dj - 1) * 128 + r * 64 + j] = 1.0

    recip = np.zeros((128, 2048), np.float32)
    for r in range(2):
        for j in range(64):
            for i2 in range(32):
                recip[r * 64 + j, i2 * 64:(i2 + 1) * 64] = 1.0 / (
                    c1[2 * i2 + r] * c1[j]
                )
    return wc, shift, recip


def _build_nc(dbg=False):
    import concourse.bacc as bacc
    import concourse.mybir as mybir
    from concourse.tile import TileContext

    f32 = mybir.dt.float32
    bf16 = mybir.dt.bfloat16
    nc = bacc.Bacc("TRN2", target_bir_lowering=False, debug=False)
    x = nc.dram_tensor("x", [PATCHES, 2 * FREE], bf16, kind="ExternalInput")
    y = nc.dram_tensor("y", [PATCHES, FREE], f32, kind="ExternalOutput")
    if dbg:
        dbg_img = nc.dram_tensor("dbg_img", [KP, 128, 2048], f32,
                                 kind="ExternalOutput")

    wc_np, shift_np, recip_np = _consts()
    import ml_dtypes
    wc_d = nc.inline_tensor(wc_np.astype(ml_dtypes.bfloat16), name="wc_c")
    shift_d = nc.inline_tensor(shift_np, name="shift_c")
    recip_d = nc.inline_tensor(recip_np, name="recip_c")

    eng_of_tile = {}
    for b in range(NT):
        k = b % 10
        if k < 5:
            eng_of_tile[b] = "dve"
        elif k < 8:
            eng_of_tile[b] = "act"
        else:
            eng_of_tile[b] = "gps"

    with TileContext(nc) as tc:
        with (
            tc.tile_pool(name="const", bufs=1) as cpool,
            tc.tile_pool(name="imgsb", bufs=1) as img_pool,
            tc.tile_pool(name="inp", bufs=4) as in_pool,
            tc.tile_pool(name="outp", bufs=6) as out_pool,
            tc.tile_pool(name="ycps", bufs=4, space="PSUM") as ycps_pool,
            tc.tile_pool(name="shps", bufs=4, space="PSUM") as shps_pool,
        ):
            wc_sb = cpool.tile([TP, 5 * 128], bf16)
            shift_sb = cpool.tile([128, 4 * 128], f32)
            recip_sb = cpool.tile([128, 2048], f32)
            nc.sync.dma_start(out=wc_sb[:], in_=wc_d[:])
            nc.sync.dma_start(out=shift_sb[:], in_=shift_d[:])
            nc.sync.dma_start(out=recip_sb[:], in_=recip_d[:])

            img_raw = img_pool.tile([128, 2048], f32)
            nc.gpsimd.memset(img_raw[:], 0.0)
            img_sb = []
            for dj in range(KP):
                t = img_pool.tile([128, 2048], f32, tag=f"img{dj}",
                                  name=f"img{dj}")
                img_sb.append(t)

            # ---- Phase 1 + 2 interleaved over tiles ----
            for b in range(NT):
                in_t = in_pool.tile([TP, 2 * FREE], bf16, tag="in_t")
                nc.sync.dma_start(out=in_t[:], in_=x[b * TP:(b + 1) * TP, :])
                yc_ps = ycps_pool.tile([128, 320], f32, tag="yc_ps")
                mm = 0
                for ej in range(KP):
                    for half in range(2):  # hi, lo
                        nc.tensor.matmul(
                            yc_ps[:, :],
                            lhsT=wc_sb[:, ej * 128:(ej + 1) * 128],
                            rhs=in_t[:, half * FREE + ej * 320:
                                     half * FREE + (ej + 1) * 320],
                            start=(mm == 0),
                            stop=(mm == 9),
                        )
                        mm += 1

                # phase 2 (DVE): windowed adds of Yc (PSUM) into img_raw
                def add_window(lo, n, src_base, dst_base, npart, ei0):
                    dst = img_raw[dst_base:dst_base + npart,
                                  lo * 64:(lo + n) * 64]
                    psrc = yc_ps[src_base:src_base + npart, :]
                    psrc = psrc.rearrange("p (ei h) -> p ei h", ei=KP)
                    s = psrc[:, ei0:KP:2, :][:, 0:n, :]
                    nc.vector.tensor_add(out=dst, in0=dst, in1=s)

                add_window(b, 3, 0, 0, 128, 0)
                for rho in (0, 1):
                    add_window(b + rho, 2, rho * 64, (1 - rho) * 64, 64, 1)

            # ---- Normalize + shifts per 512-col quarter ----
            for q in range(4):
                ncol = slice(q * 512, (q + 1) * 512)
                nc.vector.tensor_mul(out=img_sb[0][:, ncol],
                                     in0=img_raw[:, ncol],
                                     in1=recip_sb[:, ncol])
                for dj in range(1, KP):
                    sh_ps = shps_pool.tile([128, 512], f32, tag="shps")
                    nc.tensor.matmul(
                        sh_ps[:],
                        lhsT=shift_sb[:, (dj - 1) * 128:dj * 128],
                        rhs=img_sb[0][:, ncol],
                        start=True,
                        stop=True,
                    )
                    deng = nc.vector if (dj + q) % 2 == 0 else nc.scalar
                    if deng is nc.vector:
                        deng.tensor_copy(out=img_sb[dj][:, ncol],
                                         in_=sh_ps[:])
                    else:
                        deng.copy(out=img_sb[dj][:, ncol], in_=sh_ps[:])

            if dbg:
                for dj in range(KP):
                    nc.sync.dma_start(out=dbg_img[dj], in_=img_sb[dj][:])

            # ---- Phase 3: unfold (25 shifted views) + DMA out ----
            for b in range(NT):
                ekind = eng_of_tile[b]
                out_t = out_pool.tile([128, FREE], f32, tag="out_t")
                de = out_t[:].rearrange("p (h d5 dj) -> p d5 h dj",
                                        h=HF, d5=KP, dj=KP)

                def copy(dst, src):
                    if ekind == "dve":
                        nc.vector.tensor_copy(out=dst, in_=src)
                    elif ekind == "act":
                        nc.scalar.copy(out=dst, in_=src)
                    else:
                        nc.gpsimd.tensor_copy(out=dst, in_=src)

                for dj in range(KP):
                    src = img_sb[dj][:].rearrange("p (i2 h) -> p i2 h",
                                                  i2=32)
                    for d5i, di in enumerate((0, 2, 4)):
                        copy(de[0:124, di, :, dj],
                             src[0:124, b + d5i, :])
                    for rho in (0, 1):
                        for d5i, di in enumerate((1, 3)):
                            copy(de[rho * 64:rho * 64 + 60, di, :, dj],
                                 src[(1 - rho) * 64:(1 - rho) * 64 + 60,
                                     b + rho + d5i, :])
                nc.scalar.dma_start(
                    out=y[b * TP:b * TP + OW, :], in_=out_t[0:OW, :]
                )
                nc.scalar.dma_start(
                    out=y[b * TP + OW:(b + 1) * TP, :],
                    in_=out_t[64:64 + OW, :],
                )

    nc.compile()
    return nc


def _get_nc():
    if "nc" not in _CACHE:
        _CACHE["nc"] = _build_nc()
    return _CACHE["nc"]


def _pack_input(x_im):
    """x_im (3600, 64, 25) f32 -> (3600, 3200) bf16 hi/lo in
    (p, {hi,lo}, ej, ei, h) order."""
    import ml_dtypes

    xr = np.ascontiguousarray(
        x_im.reshape(PATCHES, HF, KP, KP).transpose(0, 3, 2, 1)
    ).reshape(PATCHES, FREE)
    hi = xr.astype(ml_dtypes.bfloat16)
    lo = (xr - hi.astype(np.float32)).astype(ml_dtypes.bfloat16)
    out = np.empty((PATCHES, 2, FREE), ml_dtypes.bfloat16)
    out[:, 0, :] = hi
    out[:, 1, :] = lo
    return out.reshape(PATCHES, 2 * FREE)


def kernel(x, pixels_h=64, pixels_w=64, **kw):
    from concourse.bass_utils import run_bass_kernel_spmd

    x = np.asarray(x, dtype=np.float32)
    assert x.shape == (IMAGES, PATCHES, HF, VF), x.shape
    nc = _get_nc()
    in_maps = [{"x": _pack_input(x[im])} for im in range(IMAGES)]
    res = run_bass_kernel_spmd(
        nc, in_maps, core_ids=list(range(IMAGES)), **kw
    )
    out = np.stack(
        [res.results[c]["y"].reshape(PATCHES, HF, VF) for c in range(IMAGES)]
    )
    if kw.get("trace"):
        kernel.last_results = res
    return out


# revision 12
# speedup vs baseline: 2.2021x; 1.1347x over previous
"""Trainium2 Bass kernel for nn_Aggregation0 (fold -> normalize -> unfold).

Per (image, hor_f) slice the op is: col2im (5x5, stride 1) of the 25
ver_f channels into a 64x64 image, divide by the overlap count, then
im2col back. The output is 25 shifted views of the folded image.
Sharding: one image per NeuronCore (8 images, 8 cores).

Host side:
  in:  x[im] is re-packed to (p, {hi,lo}, ej, ei, h) bf16 where
       x = hi + lo (exact to ~1e-5 rel). Phase-1 rhs becomes contiguous
       and input DMA bytes halve.
  out: kernel writes y in (p, dj, dislot, h) order with dislot =
       (di 0,2,4 | di 1,3); the host un-permutes. This makes every
       unfold copy fully contiguous on both sides.

Per core:
  Phase 1 (PE, bf16): per 120-partition tile (2 qi rows of the 60x60
    patch grid), contract qj with 5 column-shift matrices (hi+lo
    accumulated in fp32 PSUM) -> Yc[(qi_r, j); (ei, h)].
  Phase 2 (DVE): windowed adds of Yc (read from PSUM) into the folded
    image img_raw[(r, j); (i2, h)] in SBUF (i = 2*i2 + r).
  Normalize (DVE): img0 = img_raw * (1/count), per 512-col quarter;
    also split img0 into bf16 hi/lo for the shift matmuls.
  Shifts (PE, bf16 hi/lo): img_dj = column-shift-by-dj of img0 for
    dj=1..4 (so unfold reads are partition-quadrant-aligned).
  Swaps (DVE/ACT/GPS): imgsw_dj[(r,j); w] = img[2w+r+1, j+dj] via a
    partition-half swap + 64-elem free shift (plain contiguous copies).
  Phase 3 (DVE/ACT/GPS): per output tile, 10 contiguous copies
    (5 dj x {even block from img_dj, odd block from imgsw_dj});
    merged 2-tile DMAs out.
"""

import numpy as np

IMAGES = 8
PATCHES = 3600
HF = 64  # hor_f
VF = 25  # ver_f = 5*5
KP = 5  # patch width
OW = 60  # output patch grid (60x60)
IH = 64  # image height/width
FREE = HF * VF  # 1600
NT = 30  # partition tiles per image
TP = 120  # partitions per tile (2 qi rows x 60 qj)

_CACHE = {}

# order of di within a dj-block of the on-device output layout
DI_ORDER = (0, 2, 4, 1, 3)


def _consts():
    c1 = np.array(
        [min(i, OW - 1) - max(i - (KP - 1), 0) + 1 for i in range(IH)],
        np.float32,
    )

    wc = np.zeros((TP, 5 * 128), np.float32)
    for ej in range(KP):
        for r in range(2):
            for qj in range(OW):
                j = qj + ej
                wc[r * OW + qj, ej * 128 + r * 64 + j] = 1.0

    shift = np.zeros((128, 4 * 128), np.float32)
    for dj in range(1, KP):
        for r in range(2):
            for j in range(IH - dj):
                shift[r * 64 + j + dj, (dj - 1) * 128 + r * 64 + j] = 1.0

    recip = np.zeros((128, 2048), np.float32)
    for r in range(2):
        for j in range(64):
            for i2 in range(32):
                recip[r * 64 + j, i2 * 64:(i2 + 1) * 64] = 1.0 / (
                    c1[2 * i2 + r] * c1[j]
                )
    return wc, shift, recip


def _build_nc():
    import concourse.bacc as bacc
    import concourse.mybir as mybir
    import ml_dtypes
    from concourse.tile import TileContext

    f32 = mybir.dt.float32
    bf16 = mybir.dt.bfloat16
    nc = bacc.Bacc("TRN2", target_bir_lowering=False, debug=False)
    x = nc.dram_tensor("x", [PATCHES, 2 * FREE], bf16, kind="ExternalInput")
    y = nc.dram_tensor("y", [PATCHES, FREE], f32, kind="ExternalOutput")

    wc_np, shift_np, recip_np = _consts()
    wc_d = nc.inline_tensor(wc_np.astype(ml_dtypes.bfloat16), name="wc_c")
    shift_d = nc.inline_tensor(shift_np.astype(ml_dtypes.bfloat16),
                               name="shift_c")
    recip_d = nc.inline_tensor(recip_np, name="recip_c")

    # phase-3 engine per tile-pair (15 pairs): dve/act/gps
    p3_eng = {}
    for tb in range(15):
        k = tb % 5
        p3_eng[tb] = "dve" if k < 2 else ("act" if k < 4 else "gps")

    with TileContext(nc) as tc:
        with (
            tc.tile_pool(name="const", bufs=1) as cpool,
            tc.tile_pool(name="imgsb", bufs=1) as img_pool,
            tc.tile_pool(name="inp", bufs=3) as in_pool,
            tc.tile_pool(name="outp", bufs=3) as out_pool,
            tc.tile_pool(name="ycps", bufs=4, space="PSUM") as ycps_pool,
            tc.tile_pool(name="shps", bufs=4, space="PSUM") as shps_pool,
        ):
            wc_sb = cpool.tile([TP, 5 * 128], bf16)
            shift_sb = cpool.tile([128, 4 * 128], bf16)
            recip_sb = cpool.tile([128, 2048], f32)
            nc.sync.dma_start(out=wc_sb[:], in_=wc_d[:])
            nc.sync.dma_start(out=shift_sb[:], in_=shift_d[:])
            nc.sync.dma_start(out=recip_sb[:], in_=recip_d[:])

            img_raw = img_pool.tile([128, 2048], f32)
            nc.gpsimd.memset(img_raw[:], 0.0)
            img0h = img_pool.tile([128, 2048], bf16)
            img0l = img_pool.tile([128, 2048], bf16)
            img_sb = []
            imgsw_sb = []
            for dj in range(KP):
                t = img_pool.tile([128, 2048], f32, tag=f"img{dj}",
                                  name=f"img{dj}")
                img_sb.append(t)
                t2 = img_pool.tile([128, 2048], f32, tag=f"imgsw{dj}",
                                   name=f"imgsw{dj}")
                imgsw_sb.append(t2)

            # ---- Phase 1 (PE) + Phase 2 (DVE), 2 tiles per DMA ----
            for bb in range(NT // 2):
                in_t = in_pool.tile([TP, 4 * FREE], bf16, tag="in_t")
                hv = x[2 * bb * TP:(2 * bb + 2) * TP, :].rearrange(
                    "(t p) f -> p t f", t=2
                )
                nc.sync.dma_start(
                    out=in_t[:].rearrange("p (t f) -> p t f", t=2), in_=hv
                )
                for t in range(2):
                    b = 2 * bb + t
                    base = t * 2 * FREE
                    yc_ps = ycps_pool.tile([128, 320], f32, tag="yc_ps")
                    mm = 0
                    for ej in range(KP):
                        for half in range(2):  # hi, lo
                            o = base + half * FREE + ej * 320
                            nc.tensor.matmul(
                                yc_ps[:, :],
                                lhsT=wc_sb[:, ej * 128:(ej + 1) * 128],
                                rhs=in_t[:, o:o + 320],
                                start=(mm == 0),
                                stop=(mm == 9),
                            )
                            mm += 1

                    # phase 2 (DVE): windowed adds of Yc into img_raw
                    def add_window(lo, n, src_base, dst_base, npart, ei0):
                        dst = img_raw[dst_base:dst_base + npart,
                                      lo * 64:(lo + n) * 64]
                        psrc = yc_ps[src_base:src_base + npart, :]
                        psrc = psrc.rearrange("p (ei h) -> p ei h", ei=KP)
                        s = psrc[:, ei0:KP:2, :][:, 0:n, :]
                        nc.vector.tensor_add(out=dst, in0=dst, in1=s)

                    add_window(b, 3, 0, 0, 128, 0)
                    for rho in (0, 1):
                        add_window(b + rho, 2, rho * 64, (1 - rho) * 64,
                                   64, 1)

            # ---- per-quarter: normalize, hi/lo split, shifts, swaps ----
            for q in range(4):
                ncol = slice(q * 512, (q + 1) * 512)
                nc.vector.tensor_mul(out=img_sb[0][:, ncol],
                                     in0=img_raw[:, ncol],
                                     in1=recip_sb[:, ncol])
                nc.vector.tensor_copy(out=img0h[:, ncol],
                                      in_=img_sb[0][:, ncol])
                nc.vector.tensor_sub(out=img0l[:, ncol],
                                     in0=img_sb[0][:, ncol],
                                     in1=img0h[:, ncol])
                for dj in range(1, KP):
                    sh_ps = shps_pool.tile([128, 512], f32, tag="shps")
                    for hi, srct in ((0, img0h), (1, img0l)):
                        nc.tensor.matmul(
                            sh_ps[:],
                            lhsT=shift_sb[:, (dj - 1) * 128:dj * 128],
                            rhs=srct[:, ncol],
                            start=(hi == 0),
                            stop=(hi == 1),
                        )
                    deng = nc.vector if (dj + q) % 2 == 0 else nc.scalar
                    if deng is nc.vector:
                        deng.tensor_copy(out=img_sb[dj][:, ncol],
                                         in_=sh_ps[:])
                    else:
                        deng.copy(out=img_sb[dj][:, ncol], in_=sh_ps[:])

            # swaps: imgsw_dj[(0,j); w] = img_dj[(1,j); w]
            #        imgsw_dj[(1,j); w] = img_dj[(0,j); w+1]
            for dj in range(KP):
                for q in range(4):
                    lo, hi_ = q * 512, (q + 1) * 512
                    ek = ("dve", "act", "gps")[(dj + q) % 3]
                    for (dpl, dph, spl, sph, soff) in (
                        (0, 64, 64, 128, 0),
                        (64, 128, 0, 64, 64),
                    ):
                        n = 512
                        if soff and q == 3:
                            n = 512 - 64
                        dst = imgsw_sb[dj][dpl:dph, lo:lo + n]
                        src = img_sb[dj][spl:sph, lo + soff:lo + soff + n]
                        if ek == "dve":
                            nc.vector.tensor_copy(out=dst, in_=src)
                        elif ek == "act":
                            nc.scalar.copy(out=dst, in_=src)
                        else:
                            nc.gpsimd.tensor_copy(out=dst, in_=src)

            # ---- Phase 3: 10 contiguous copies per tile + merged DMAs ----
            for tb in range(NT // 2):
                ekind = p3_eng[tb]
                out_t = out_pool.tile([128, 2 * FREE], f32, tag="out_t")

                def copy(dst, src):
                    if ekind == "dve":
                        nc.vector.tensor_copy(out=dst, in_=src)
                    elif ekind == "act":
                        nc.scalar.copy(out=dst, in_=src)
                    else:
                        nc.gpsimd.tensor_copy(out=dst, in_=src)

                for t in range(2):
                    b = 2 * tb + t
                    base = t * FREE
                    for dj in range(KP):
                        o = base + dj * 5 * 64
                        copy(out_t[0:124, o:o + 192],
                             img_sb[dj][0:124, b * 64:(b + 3) * 64])
                        copy(out_t[0:124, o + 192:o + 320],
                             imgsw_sb[dj][0:124, b * 64:(b + 2) * 64])
                # stores: r0-halves then r1-halves of both b's
                yv = y[2 * tb * TP:(2 * tb + 2) * TP, :].rearrange(
                    "(b2 r p) f -> r p b2 f", b2=2, r=2
                )
                sv = out_t[:].rearrange("p (b2 f) -> p b2 f", b2=2)
                nc.scalar.dma_start(out=yv[0], in_=sv[0:OW])
                nc.scalar.dma_start(out=yv[1], in_=sv[64:64 + OW])

    nc.compile()
    return nc


def _get_nc():
    if "nc" not in _CACHE:
        _CACHE["nc"] = _build_nc()
    return _CACHE["nc"]


def _pack_input(x_im):
    """x_im (3600, 64, 25) f32 -> (3600, 3200) bf16 hi/lo in
    (p, {hi,lo}, ej, ei, h) order."""
    import ml_dtypes

    xr = np.ascontiguousarray(
        x_im.reshape(PATCHES, HF, KP, KP).transpose(0, 3, 2, 1)
    ).reshape(PATCHES, FREE)
    hi = xr.astype(ml_dtypes.bfloat16)
    lo = (xr - hi.astype(np.float32)).astype(ml_dtypes.bfloat16)
    out = np.empty((PATCHES, 2, FREE), ml_dtypes.bfloat16)
    out[:, 0, :] = hi
    out[:, 1, :] = lo
    return out.reshape(PATCHES, 2 * FREE)


def _unpack_output(y_im):
    """y_im (3600, 1600) in (p, dj, dislot, h) -> (3600, 64, 25)."""
    arr = y_im.reshape(PATCHES, KP, KP, HF)  # (p, dj, slot, h)
    slot_of_di = [DI_ORDER.index(di) for di in range(KP)]
    tmp = arr[:, :, slot_of_di, :]  # (p, dj, di, h)
    return np.ascontiguousarray(tmp.transpose(0, 3, 2, 1)).reshape(
        PATCHES, HF, VF
    )


def kernel(x, pixels_h=64, pixels_w=64, **kw):
    from concourse.bass_utils import run_bass_kernel_spmd

    x = np.asarray(x, dtype=np.float32)
    assert x.shape == (IMAGES, PATCHES, HF, VF), x.shape
    nc = _get_nc()
    in_maps = [{"x": _pack_input(x[im])} for im in range(IMAGES)]
    res = run_bass_kernel_spmd(
        nc, in_maps, core_ids=list(range(IMAGES)), **kw
    )
    out = np.stack(
        [_unpack_output(res.results[c]["y"]) for c in range(IMAGES)]
    )
    if kw.get("trace"):
        kernel.last_results = res
    return out


# revision 13
# speedup vs baseline: 2.3388x; 1.0621x over previous
"""Trainium2 Bass kernel for nn_Aggregation0 (fold -> normalize -> unfold).

Per (image, hor_f) slice the op is: col2im (5x5, stride 1) of the 25
ver_f channels into a 64x64 image, divide by the overlap count, then
im2col back. The output is 25 shifted views of the folded image.
Sharding: one image per NeuronCore (8 images, 8 cores).

Host side:
  in:  x[im] is re-packed to (p, {hi,lo}, ej, ei, h) bf16 where
       x = hi + lo (exact to ~1e-5 rel). Phase-1 rhs becomes contiguous
       and input DMA bytes halve.
  out: kernel writes y in (p, dj, dislot, h) order with dislot =
       (di 0,2,4 | di 1,3); the host un-permutes. This makes every
       unfold copy fully contiguous on both sides.

Per core:
  Phase 1 (PE, bf16): per 120-partition tile (2 qi rows of the 60x60
    patch grid), contract qj with 5 column-shift matrices (hi+lo
    accumulated in fp32 PSUM) -> Yc[(qi_r, j); (ei, h)].
  Phase 2 (DVE): windowed adds of Yc (read from PSUM) into the folded
    image img_raw[(r, j); (i2, h)] in SBUF (i = 2*i2 + r).
  Normalize (DVE): img0 = img_raw * (1/count), per 512-col quarter;
    also split img0 into bf16 hi/lo for the shift matmuls.
  Shifts (PE, bf16 hi/lo): img_dj = column-shift-by-dj of img0 for
    dj=1..4 (so unfold reads are partition-quadrant-aligned).
  Swaps (DVE/ACT/GPS): imgsw_dj[(r,j); w] = img[2w+r+1, j+dj] via a
    partition-half swap + 64-elem free shift (plain contiguous copies).
  Phase 3 (DVE/ACT/GPS): per output tile, 10 contiguous copies
    (5 dj x {even block from img_dj, odd block from imgsw_dj});
    merged 2-tile DMAs out.
"""

import numpy as np

IMAGES = 8
PATCHES = 3600
HF = 64  # hor_f
VF = 25  # ver_f = 5*5
KP = 5  # patch width
OW = 60  # output patch grid (60x60)
IH = 64  # image height/width
FREE = HF * VF  # 1600
NT = 30  # partition tiles per image
TP = 120  # partitions per tile (2 qi rows x 60 qj)

_CACHE = {}

# order of di within a dj-block of the on-device output layout
DI_ORDER = (0, 2, 4, 1, 3)


def _consts():
    c1 = np.array(
        [min(i, OW - 1) - max(i - (KP - 1), 0) + 1 for i in range(IH)],
        np.float32,
    )

    wc = np.zeros((TP, 5 * 128), np.float32)
    for ej in range(KP):
        for r in range(2):
            for qj in range(OW):
                j = qj + ej
                wc[r * OW + qj, ej * 128 + r * 64 + j] = 1.0

    shift = np.zeros((128, 4 * 128), np.float32)
    for dj in range(1, KP):
        for r in range(2):
            for j in range(IH - dj):
                shift[r * 64 + j + dj, (dj - 1) * 128 + r * 64 + j] = 1.0

    recip = np.zeros((128, 2048), np.float32)
    for r in range(2):
        for j in range(64):
            for i2 in range(32):
                recip[r * 64 + j, i2 * 64:(i2 + 1) * 64] = 1.0 / (
                    c1[2 * i2 + r] * c1[j]
                )
    return wc, shift, recip


def _build_nc():
    import concourse.bacc as bacc
    import concourse.mybir as mybir
    import ml_dtypes
    from concourse.tile import TileContext

    f32 = mybir.dt.float32
    bf16 = mybir.dt.bfloat16
    nc = bacc.Bacc("TRN2", target_bir_lowering=False, debug=False)
    x = nc.dram_tensor("x", [PATCHES, 2 * FREE], bf16, kind="ExternalInput")
    y = nc.dram_tensor("y", [PATCHES, FREE], f32, kind="ExternalOutput")

    wc_np, shift_np, recip_np = _consts()
    wc_d = nc.inline_tensor(wc_np.astype(ml_dtypes.bfloat16), name="wc_c")
    shift_d = nc.inline_tensor(shift_np.astype(ml_dtypes.bfloat16),
                               name="shift_c")
    recip_d = nc.inline_tensor(recip_np, name="recip_c")

    # phase-3 engine per tile-pair (15 pairs): dve/act/gps
    p3_eng = {}
    for tb in range(15):
        p3_eng[tb] = "dve" if tb % 2 == 0 else "act"

    with TileContext(nc) as tc:
        with (
            tc.tile_pool(name="const", bufs=1) as cpool,
            tc.tile_pool(name="imgsb", bufs=1) as img_pool,
            tc.tile_pool(name="inp", bufs=3) as in_pool,
            tc.tile_pool(name="outp", bufs=3) as out_pool,
            tc.tile_pool(name="ycps", bufs=6, space="PSUM") as ycps_pool,
            tc.tile_pool(name="shps", bufs=2, space="PSUM") as shps_pool,
        ):
            wc_sb = cpool.tile([TP, 5 * 128], bf16)
            shift_sb = cpool.tile([128, 4 * 128], bf16)
            recip_sb = cpool.tile([128, 2048], f32)
            nc.sync.dma_start(out=wc_sb[:], in_=wc_d[:])
            nc.sync.dma_start(out=shift_sb[:], in_=shift_d[:])
            nc.sync.dma_start(out=recip_sb[:], in_=recip_d[:])

            img_raw = img_pool.tile([128, 2048], f32)
            nc.gpsimd.memset(img_raw[:], 0.0)
            img0h = img_pool.tile([128, 2048], bf16)
            img0l = img_pool.tile([128, 2048], bf16)
            img_sb = []
            imgsw_sb = []
            for dj in range(KP):
                t = img_pool.tile([128, 2048], f32, tag=f"img{dj}",
                                  name=f"img{dj}")
                img_sb.append(t)
                t2 = img_pool.tile([128, 2048], f32, tag=f"imgsw{dj}",
                                   name=f"imgsw{dj}")
                imgsw_sb.append(t2)

            # ---- Phase 1 (PE) + Phase 2 (DVE), 2 tiles per DMA ----
            for bb in range(NT // 2):
                in_t = in_pool.tile([TP, 4 * FREE], bf16, tag="in_t")
                hv = x[2 * bb * TP:(2 * bb + 2) * TP, :].rearrange(
                    "(t p) f -> p t f", t=2
                )
                nc.sync.dma_start(
                    out=in_t[:].rearrange("p (t f) -> p t f", t=2), in_=hv
                )
                for t in range(2):
                    b = 2 * bb + t
                    base = t * 2 * FREE
                    yc_ps = ycps_pool.tile([128, 320], f32, tag="yc_ps")
                    mm = 0
                    for ej in range(KP):
                        for half in range(2):  # hi, lo
                            o = base + half * FREE + ej * 320
                            nc.tensor.matmul(
                                yc_ps[:, :],
                                lhsT=wc_sb[:, ej * 128:(ej + 1) * 128],
                                rhs=in_t[:, o:o + 320],
                                start=(mm == 0),
                                stop=(mm == 9),
                            )
                            mm += 1

                    # phase 2 (DVE): windowed adds of Yc into img_raw
                    def add_window(lo, n, src_base, dst_base, npart, ei0):
                        dst = img_raw[dst_base:dst_base + npart,
                                      lo * 64:(lo + n) * 64]
                        psrc = yc_ps[src_base:src_base + npart, :]
                        psrc = psrc.rearrange("p (ei h) -> p ei h", ei=KP)
                        s = psrc[:, ei0:KP:2, :][:, 0:n, :]
                        nc.vector.tensor_add(out=dst, in0=dst, in1=s)

                    add_window(b, 3, 0, 0, 128, 0)
                    for rho in (0, 1):
                        add_window(b + rho, 2, rho * 64, (1 - rho) * 64,
                                   64, 1)

            # ---- per-quarter: normalize, hi/lo split, shifts, swaps ----
            for q in range(4):
                ncol = slice(q * 512, (q + 1) * 512)
                nc.vector.tensor_mul(out=img_sb[0][:, ncol],
                                     in0=img_raw[:, ncol],
                                     in1=recip_sb[:, ncol])
                nc.vector.tensor_copy(out=img0h[:, ncol],
                                      in_=img_sb[0][:, ncol])
                nc.vector.tensor_sub(out=img0l[:, ncol],
                                     in0=img_sb[0][:, ncol],
                                     in1=img0h[:, ncol])
                for dj in range(1, KP):
                    sh_ps = shps_pool.tile([128, 512], f32, tag="shps")
                    for hi, srct in ((0, img0h), (1, img0l)):
                        nc.tensor.matmul(
                            sh_ps[:],
                            lhsT=shift_sb[:, (dj - 1) * 128:dj * 128],
                            rhs=srct[:, ncol],
                            start=(hi == 0),
                            stop=(hi == 1),
                        )
                    deng = nc.vector if (dj + q) % 2 == 0 else nc.scalar
                    if deng is nc.vector:
                        deng.tensor_copy(out=img_sb[dj][:, ncol],
                                         in_=sh_ps[:])
                    else:
                        deng.copy(out=img_sb[dj][:, ncol], in_=sh_ps[:])

            # swaps: imgsw_dj[(0,j); w] = img_dj[(1,j); w]
            #        imgsw_dj[(1,j); w] = img_dj[(0,j); w+1]
            for dj in range(KP):
                for q in range(4):
                    lo, hi_ = q * 512, (q + 1) * 512
                    ek = "gps"
                    for (dpl, dph, spl, sph, soff) in (
                        (0, 64, 64, 128, 0),
                        (64, 128, 0, 64, 64),
                    ):
                        n = 512
                        if soff and q == 3:
                            n = 512 - 64
                        dst = imgsw_sb[dj][dpl:dph, lo:lo + n]
                        src = img_sb[dj][spl:sph, lo + soff:lo + soff + n]
                        if ek == "dve":
                            nc.vector.tensor_copy(out=dst, in_=src)
                        elif ek == "act":
                            nc.scalar.copy(out=dst, in_=src)
                        else:
                            nc.gpsimd.tensor_copy(out=dst, in_=src)

            # ---- Phase 3: 10 contiguous copies per tile + merged DMAs ----
            for tb in range(NT // 2):
                ekind = p3_eng[tb]
                out_t = out_pool.tile([128, 2 * FREE], f32, tag="out_t")

                def copy(dst, src):
                    if ekind == "dve":
                        nc.vector.tensor_copy(out=dst, in_=src)
                    elif ekind == "act":
                        nc.scalar.copy(out=dst, in_=src)
                    else:
                        nc.gpsimd.tensor_copy(out=dst, in_=src)

                for t in range(2):
                    b = 2 * tb + t
                    base = t * FREE
                    for dj in range(KP):
                        o = base + dj * 5 * 64
                        copy(out_t[0:124, o:o + 192],
                             img_sb[dj][0:124, b * 64:(b + 3) * 64])
                        copy(out_t[0:124, o + 192:o + 320],
                             imgsw_sb[dj][0:124, b * 64:(b + 2) * 64])
                # stores: r0-halves then r1-halves of both b's
                yv = y[2 * tb * TP:(2 * tb + 2) * TP, :].rearrange(
                    "(b2 r p) f -> r p b2 f", b2=2, r=2
                )
                sv = out_t[:].rearrange("p (b2 f) -> p b2 f", b2=2)
                nc.scalar.dma_start(out=yv[0], in_=sv[0:OW])
                nc.sync.dma_start(out=yv[1], in_=sv[64:64 + OW])

    nc.compile()
    return nc


def _get_nc():
    if "nc" not in _CACHE:
        _CACHE["nc"] = _build_nc()
    return _CACHE["nc"]


def _pack_input(x_im):
    """x_im (3600, 64, 25) f32 -> (3600, 3200) bf16 hi/lo in
    (p, {hi,lo}, ej, ei, h) order."""
    import ml_dtypes

    xr = np.ascontiguousarray(
        x_im.reshape(PATCHES, HF, KP, KP).transpose(0, 3, 2, 1)
    ).reshape(PATCHES, FREE)
    hi = xr.astype(ml_dtypes.bfloat16)
    lo = (xr - hi.astype(np.float32)).astype(ml_dtypes.bfloat16)
    out = np.empty((PATCHES, 2, FREE), ml_dtypes.bfloat16)
    out[:, 0, :] = hi
    out[:, 1, :] = lo
    return out.reshape(PATCHES, 2 * FREE)


def _unpack_output(y_im):
    """y_im (3600, 1600) in (p, dj, dislot, h) -> (3600, 64, 25)."""
    arr = y_im.reshape(PATCHES, KP, KP, HF)  # (p, dj, slot, h)
    slot_of_di = [DI_ORDER.index(di) for di in range(KP)]
    tmp = arr[:, :, slot_of_di, :]  # (p, dj, di, h)
    return np.ascontiguousarray(tmp.transpose(0, 3, 2, 1)).reshape(
        PATCHES, HF, VF
    )


def kernel(x, pixels_h=64, pixels_w=64, **kw):
    from concourse.bass_utils import run_bass_kernel_spmd

    x = np.asarray(x, dtype=np.float32)
    assert x.shape == (IMAGES, PATCHES, HF, VF), x.shape
    nc = _get_nc()
    in_maps = [{"x": _pack_input(x[im])} for im in range(IMAGES)]
    res = run_bass_kernel_spmd(
        nc, in_maps, core_ids=list(range(IMAGES)), **kw
    )
    out = np.stack(
        [_unpack_output(res.results[c]["y"]) for c in range(IMAGES)]
    )
    if kw.get("trace"):
        kernel.last_results = res
    return out


# revision 14
# speedup vs baseline: 2.4857x; 1.0628x over previous
"""Trainium2 Bass kernel for nn_Aggregation0 (fold -> normalize -> unfold).

Per (image, hor_f) slice the op is: col2im (5x5, stride 1) of the 25
ver_f channels into a 64x64 image, divide by the overlap count, then
im2col back. The output is 25 shifted views of the folded image.
Sharding: one image per NeuronCore (8 images, 8 cores).

Host side:
  in:  x[im] is re-packed to (p, {hi,lo}, ej, ei, h) bf16 where
       x = hi + lo (exact to ~1e-5 rel). Phase-1 rhs becomes contiguous
       and input DMA bytes halve.
  out: kernel writes y in (p, dj, dislot, h) order with dislot =
       (di 0,2,4 | di 1,3); the host un-permutes. This makes every
       unfold copy fully contiguous on both sides.

Per core:
  Phase 1 (PE, bf16): per 120-partition tile (2 qi rows of the 60x60
    patch grid), contract qj with 5 column-shift matrices (hi+lo
    accumulated in fp32 PSUM) -> Yc[(qi_r, j); (ei, h)].
  Phase 2 (DVE): windowed adds of Yc (read from PSUM) into the folded
    image img_raw[(r, j); (i2, h)] in SBUF (i = 2*i2 + r).
  Normalize (DVE): img0 = img_raw * (1/count), per 512-col quarter;
    also split img0 into bf16 hi/lo for the shift matmuls.
  Shifts (PE, bf16 hi/lo): img_dj = column-shift-by-dj of img0 for
    dj=1..4 (so unfold reads are partition-quadrant-aligned).
  Swaps (DVE/ACT/GPS): imgsw_dj[(r,j); w] = img[2w+r+1, j+dj] via a
    partition-half swap + 64-elem free shift (plain contiguous copies).
  Phase 3 (DVE/ACT/GPS): per output tile, 10 contiguous copies
    (5 dj x {even block from img_dj, odd block from imgsw_dj});
    merged 2-tile DMAs out.
"""

import numpy as np

IMAGES = 8
PATCHES = 3600
HF = 64  # hor_f
VF = 25  # ver_f = 5*5
KP = 5  # patch width
OW = 60  # output patch grid (60x60)
IH = 64  # image height/width
FREE = HF * VF  # 1600
NT = 30  # partition tiles per image
TP = 120  # partitions per tile (2 qi rows x 60 qj)

_CACHE = {}

# order of di within a dj-block of the on-device output layout
DI_ORDER = (0, 2, 4, 1, 3)


def _consts():
    c1 = np.array(
        [min(i, OW - 1) - max(i - (KP - 1), 0) + 1 for i in range(IH)],
        np.float32,
    )

    wc = np.zeros((TP, 5 * 128), np.float32)
    for ej in range(KP):
        for r in range(2):
            for qj in range(OW):
                j = qj + ej
                wc[r * OW + qj, ej * 128 + r * 64 + j] = 1.0

    shift = np.zeros((128, 4 * 128), np.float32)
    for dj in range(1, KP):
        for r in range(2):
            for j in range(IH - dj):
                shift[r * 64 + j + dj, (dj - 1) * 128 + r * 64 + j] = 1.0

    recip = np.zeros((128, 2048), np.float32)
    for r in range(2):
        for j in range(64):
            for i2 in range(32):
                recip[r * 64 + j, i2 * 64:(i2 + 1) * 64] = 1.0 / (
                    c1[2 * i2 + r] * c1[j]
                )
    return wc, shift, recip


def _build_nc():
    import concourse.bacc as bacc
    import concourse.mybir as mybir
    import ml_dtypes
    from concourse.tile import TileContext

    f32 = mybir.dt.float32
    bf16 = mybir.dt.bfloat16
    nc = bacc.Bacc("TRN2", target_bir_lowering=False, debug=False)
    x = nc.dram_tensor("x", [PATCHES, 2 * FREE], bf16, kind="ExternalInput")
    y = nc.dram_tensor("y", [PATCHES, FREE], f32, kind="ExternalOutput")

    wc_np, shift_np, recip_np = _consts()
    wc_d = nc.inline_tensor(wc_np.astype(ml_dtypes.bfloat16), name="wc_c")
    shift_d = nc.inline_tensor(shift_np.astype(ml_dtypes.bfloat16),
                               name="shift_c")
    recip_d = nc.inline_tensor(recip_np, name="recip_c")

    # phase-3 engine per tile-pair (15 pairs): dve/act/gps
    p3_eng = {}
    for tb in range(15):
        p3_eng[tb] = "act" if tb < 9 else "dve"

    with TileContext(nc) as tc:
        with (
            tc.tile_pool(name="const", bufs=1) as cpool,
            tc.tile_pool(name="imgsb", bufs=1) as img_pool,
            tc.tile_pool(name="inp", bufs=3) as in_pool,
            tc.tile_pool(name="outp", bufs=3) as out_pool,
            tc.tile_pool(name="ycps", bufs=6, space="PSUM") as ycps_pool,
            tc.tile_pool(name="shps", bufs=2, space="PSUM") as shps_pool,
        ):
            wc_sb = cpool.tile([TP, 5 * 128], bf16)
            shift_sb = cpool.tile([128, 4 * 128], bf16)
            recip_sb = cpool.tile([128, 2048], f32)
            nc.sync.dma_start(out=wc_sb[:], in_=wc_d[:])
            nc.sync.dma_start(out=shift_sb[:], in_=shift_d[:])
            nc.sync.dma_start(out=recip_sb[:], in_=recip_d[:])

            img_raw = img_pool.tile([128, 2048], f32)
            nc.gpsimd.memset(img_raw[:], 0.0)
            img0h = img_pool.tile([128, 2048], bf16)
            img0l = img_pool.tile([128, 2048], bf16)
            img_sb = []
            imgsw_sb = []
            for dj in range(KP):
                t = img_pool.tile([128, 2048], f32, tag=f"img{dj}",
                                  name=f"img{dj}")
                img_sb.append(t)
                t2 = img_pool.tile([128, 2048], f32, tag=f"imgsw{dj}",
                                   name=f"imgsw{dj}")
                imgsw_sb.append(t2)

            # ---- Phase 1 (PE) + Phase 2 (DVE), 2 tiles per DMA ----
            for bb in range(NT // 2):
                in_t = in_pool.tile([TP, 4 * FREE], bf16, tag="in_t")
                hv = x[2 * bb * TP:(2 * bb + 2) * TP, :].rearrange(
                    "(t p) f -> p t f", t=2
                )
                nc.sync.dma_start(
                    out=in_t[:].rearrange("p (t f) -> p t f", t=2), in_=hv
                )
                for t in range(2):
                    b = 2 * bb + t
                    base = t * 2 * FREE
                    yc_ps = ycps_pool.tile([128, 320], f32, tag="yc_ps")
                    mm = 0
                    for ej in range(KP):
                        for half in range(2):  # hi, lo
                            o = base + half * FREE + ej * 320
                            nc.tensor.matmul(
                                yc_ps[:, :],
                                lhsT=wc_sb[:, ej * 128:(ej + 1) * 128],
                                rhs=in_t[:, o:o + 320],
                                start=(mm == 0),
                                stop=(mm == 9),
                            )
                            mm += 1

                    # phase 2 (DVE): windowed adds of Yc into img_raw
                    def add_window(lo, n, src_base, dst_base, npart, ei0):
                        dst = img_raw[dst_base:dst_base + npart,
                                      lo * 64:(lo + n) * 64]
                        psrc = yc_ps[src_base:src_base + npart, :]
                        psrc = psrc.rearrange("p (ei h) -> p ei h", ei=KP)
                        s = psrc[:, ei0:KP:2, :][:, 0:n, :]
                        nc.vector.tensor_add(out=dst, in0=dst, in1=s)

                    add_window(b, 3, 0, 0, 128, 0)
                    for rho in (0, 1):
                        add_window(b + rho, 2, rho * 64, (1 - rho) * 64,
                                   64, 1)

            # ---- per-quarter: normalize, hi/lo split, shifts, swaps ----
            for q in range(4):
                ncol = slice(q * 512, (q + 1) * 512)
                nc.vector.tensor_mul(out=img_sb[0][:, ncol],
                                     in0=img_raw[:, ncol],
                                     in1=recip_sb[:, ncol])
                nc.vector.tensor_copy(out=img0h[:, ncol],
                                      in_=img_sb[0][:, ncol])
                nc.vector.tensor_sub(out=img0l[:, ncol],
                                     in0=img_sb[0][:, ncol],
                                     in1=img0h[:, ncol])
                for dj in range(1, KP):
                    sh_ps = shps_pool.tile([128, 512], f32, tag="shps")
                    for hi, srct in ((0, img0h), (1, img0l)):
                        nc.tensor.matmul(
                            sh_ps[:],
                            lhsT=shift_sb[:, (dj - 1) * 128:dj * 128],
                            rhs=srct[:, ncol],
                            start=(hi == 0),
                            stop=(hi == 1),
                        )
                    nc.scalar.copy(out=img_sb[dj][:, ncol], in_=sh_ps[:])

            # swaps: imgsw_dj[(0,j); w] = img_dj[(1,j); w]
            #        imgsw_dj[(1,j); w] = img_dj[(0,j); w+1]
            for dj in range(KP):
                for q in range(4):
                    lo, hi_ = q * 512, (q + 1) * 512
                    ek = "act"
                    for (dpl, dph, spl, sph, soff) in (
                        (0, 64, 64, 128, 0),
                        (64, 128, 0, 64, 64),
                    ):
                        n = 512
                        if soff and q == 3:
                            n = 512 - 64
                        dst = imgsw_sb[dj][dpl:dph, lo:lo + n]
                        src = img_sb[dj][spl:sph, lo + soff:lo + soff + n]
                        if ek == "dve":
                            nc.vector.tensor_copy(out=dst, in_=src)
                        elif ek == "act":
                            nc.scalar.copy(out=dst, in_=src)
                        else:
                            nc.gpsimd.tensor_copy(out=dst, in_=src)

            # ---- Phase 3: 10 contiguous copies per tile + merged DMAs ----
            for tb in range(NT // 2):
                ekind = p3_eng[tb]
                out_t = out_pool.tile([128, 2 * FREE], f32, tag="out_t")

                def copy(dst, src):
                    if ekind == "dve":
                        nc.vector.tensor_copy(out=dst, in_=src)
                    elif ekind == "act":
                        nc.scalar.copy(out=dst, in_=src)
                    else:
                        nc.gpsimd.tensor_copy(out=dst, in_=src)

                for t in range(2):
                    b = 2 * tb + t
                    base = t * FREE
                    for dj in range(KP):
                        o = base + dj * 5 * 64
                        copy(out_t[0:124, o:o + 192],
                             img_sb[dj][0:124, b * 64:(b + 3) * 64])
                        copy(out_t[0:124, o + 192:o + 320],
                             imgsw_sb[dj][0:124, b * 64:(b + 2) * 64])
                # stores: r0-halves then r1-halves of both b's
                yv = y[2 * tb * TP:(2 * tb + 2) * TP, :].rearrange(
                    "(b2 r p) f -> r p b2 f", b2=2, r=2
                )
                sv = out_t[:].rearrange("p (b2 f) -> p b2 f", b2=2)
                nc.sync.dma_start(out=yv[0], in_=sv[0:OW])
                nc.sync.dma_start(out=yv[1], in_=sv[64:64 + OW])

    nc.compile()
    return nc


def _get_nc():
    if "nc" not in _CACHE:
        _CACHE["nc"] = _build_nc()
    return _CACHE["nc"]


def _pack_input(x_im):
    """x_im (3600, 64, 25) f32 -> (3600, 3200) bf16 hi/lo in
    (p, {hi,lo}, ej, ei, h) order."""
    import ml_dtypes

    xr = np.ascontiguousarray(
        x_im.reshape(PATCHES, HF, KP, KP).transpose(0, 3, 2, 1)
    ).reshape(PATCHES, FREE)
    hi = xr.astype(ml_dtypes.bfloat16)
    lo = (xr - hi.astype(np.float32)).astype(ml_dtypes.bfloat16)
    out = np.empty((PATCHES, 2, FREE), ml_dtypes.bfloat16)
    out[:, 0, :] = hi
    out[:, 1, :] = lo
    return out.reshape(PATCHES, 2 * FREE)


def _unpack_output(y_im):
    """y_im (3600, 1600) in (p, dj, dislot, h) -> (3600, 64, 25)."""
    arr = y_im.reshape(PATCHES, KP, KP, HF)  # (p, dj, slot, h)
    slot_of_di = [DI_ORDER.index(di) for di in range(KP)]
    tmp = arr[:, :, slot_of_di, :]  # (p, dj, di, h)
    return np.ascontiguousarray(tmp.transpose(0, 3, 2, 1)).reshape(
        PATCHES, HF, VF
    )


def kernel(x, pixels_h=64, pixels_w=64, **kw):
    from concourse.bass_utils import run_bass_kernel_spmd

    x = np.asarray(x, dtype=np.float32)
    assert x.shape == (IMAGES, PATCHES, HF, VF), x.shape
    nc = _get_nc()
    in_maps = [{"x": _pack_input(x[im])} for im in range(IMAGES)]
    res = run_bass_kernel_spmd(
        nc, in_maps, core_ids=list(range(IMAGES)), **kw
    )
    out = np.stack(
        [_unpack_output(res.results[c]["y"]) for c in range(IMAGES)]
    )
    if kw.get("trace"):
        kernel.last_results = res
    return out
